# revision 1
# baseline (speedup 1.0000x reference)
"""Block-sparse attention Trainium2 kernel (8 NeuronCores, SPMD).

Sharding: data-parallel over (batch, head-group): core c handles batch b=c//4
and heads [4*(c%4) .. 4*(c%4)+4). Block index lists are replicated (used
host-side to build the static program). Each core returns a partial
[S, E] output (its heads' contribution through Wo); the host sums the 4
partials per batch (the unshard step of the head-sharded GEMM).

Pipeline per core (all on device):
  x -> PE-transpose -> x^T -> QKV projection (weights stationary) giving
  Q^T,K^T [d,s] (f32) and V^T -> PE-transpose -> V [s,d] (bf16).
  Per head-pair (2 heads packed on 128 partitions):
    Phase A (per row-block i): scores = Q_i^T.T @ K^T runs (PSUM) -> exp (ACT)
      -> per-block denom (DVE 3D reduce) -> recip -> normalize (GPSIMD, bf16)
      -> PE-transpose 64x64 blocks -> attnT storage (bf16).
    Phase B: out^T accumulated in PSUM via V-stationary matmuls over attnT.
  Wo projection from out^T tiles (stationary) + rank-1 bias add; partial out
  DMA'd back.
"""
import numpy as np

B, S, E, H, D, BS = 2, 2048, 1024, 16, 64, 64
NB = S // BS          # 32
NCORES = 8
HPC = 4               # heads per core
NPAIRS = 628          # length of block index lists

LAST_RESULTS = None   # BassKernelResults of the most recent run (for test.py)


# ---------------------------------------------------------------- host planning

def _plan(block_rows, block_cols):
    """Static schedule shared by every head/core.

    Returns dict with per-row-block structures:
      J[i]            sorted col-block list
      entries[i]      list of ('pair', j_even) / ('single', j) in storage order
                      (pairs first, then singles-even, then singles-odd, grouped
                      per score-chunk)
      score_chunks[i] list of (jlist,) groups of <=8 blocks, pair-aligned
      col[i][e]       attnT column (in 64-col units) of entry index e
      ncols_total     total attnT 64-col slots per head
    """
    J = [[] for _ in range(NB)]
    for r, c in zip(np.asarray(block_rows).tolist(), np.asarray(block_cols).tolist()):
        J[int(r)].append(int(c))
    for i in range(NB):
        J[i].sort()

    entries = [[] for _ in range(NB)]       # storage order per i
    score_chunks = [[] for _ in range(NB)]  # list of dicts
    for i in range(NB):
        js = J[i]
        # walk -> chunk entries (pair / single), in j order
        walk = []
        t = 0
        while t < len(js):
            if t + 1 < len(js) and js[t] % 2 == 0 and js[t + 1] == js[t] + 1:
                walk.append(('pair', js[t]))
                t += 2
            else:
                walk.append(('single', js[t]))
                t += 1
        # score chunks: consecutive walk entries, <=8 j-blocks each
        sc_list = []
        cur, cur_blocks = [], 0
        for ent in walk:
            nb = 2 if ent[0] == 'pair' else 1
            if cur_blocks + nb > 8:
                sc_list.append(cur)
                cur, cur_blocks = [], 0
            cur.append(ent)
            cur_blocks += nb
        if cur:
            sc_list.append(cur)
        # per score chunk: reorder storage as pairs, singles-even, singles-odd
        for sc in sc_list:
            jlist = []
            for ent in sc:
                jlist.append(ent[1])
                if ent[0] == 'pair':
                    jlist.append(ent[1] + 1)
            pairs = [e for e in sc if e[0] == 'pair']
            sE = [e for e in sc if e[0] == 'single' and e[1] % 2 == 0]
            sO = [e for e in sc if e[0] == 'single' and e[1] % 2 == 1]
            score_chunks[i].append(dict(jlist=jlist, pairs=pairs, sE=sE, sO=sO,
                                        e0=len(entries[i])))
            entries[i].extend(pairs + sE + sO)

    col = [dict() for _ in range(NB)]
    ncols = 0
    for i in range(NB):
        for e_idx, ent in enumerate(entries[i]):
            col[i][e_idx] = ncols
            ncols += 1
    return dict(J=J, entries=entries, score_chunks=score_chunks, col=col,
                ncols_total=ncols)


def _runs(jlist):
    """Maximal consecutive runs [(j0, n), ...] in a sorted j list."""
    runs = []
    for j in jlist:
        if runs and j == runs[-1][0] + runs[-1][1]:
            runs[-1][1] += 1
        else:
            runs.append([j, 1])
    return [(a, b) for a, b in runs]


# ---------------------------------------------------------------- bass program

def _build_program(plan, stage='full'):
    import concourse.bacc as bacc
    import concourse.mybir as mybir
    from concourse.tile import TileContext
    from concourse import masks

    F32 = mybir.dt.float32
    BF16 = mybir.dt.bfloat16
    AF = mybir.ActivationFunctionType
    ALU = mybir.AluOpType
    AX = mybir.AxisListType

    nc = bacc.Bacc("TRN2", target_bir_lowering=False, debug=False)

    x_in = nc.dram_tensor("x_local", [S, E], F32, kind="ExternalInput")
    wqkv_in = nc.dram_tensor("w_qkv", [E, 3 * HPC * D], F32, kind="ExternalInput")
    bqkv_in = nc.dram_tensor("b_qkv", [3 * HPC * D], F32, kind="ExternalInput")
    wo_in = nc.dram_tensor("w_o", [HPC * D, E], F32, kind="ExternalInput")
    bo_in = nc.dram_tensor("b_o", [E], F32, kind="ExternalInput")
    y_out = nc.dram_tensor("y_partial", [S, E], F32, kind="ExternalOutput")

    NT = 3 * HPC * D // 128      # 6 qkv n-tiles
    KT = E // 128                # 8 contraction tiles
    ST = S // 128                # 16 s tiles

    ncols_total = plan['ncols_total']
    entries, score_chunks, col = plan['entries'], plan['score_chunks'], plan['col']
    J = plan['J']

    with TileContext(nc) as tc:
        with tc.tile_pool(name="const", bufs=1) as cpool, \
             tc.tile_pool(name="qkvT", bufs=1) as qpool, \
             tc.tile_pool(name="vnorm", bufs=1) as vpool, \
             tc.tile_pool(name="outsb", bufs=1) as opool, \
             tc.tile_pool(name="mm_ps", bufs=2, space="PSUM") as ps_mm, \
             tc.tile_pool(name="tr_ps", bufs=2, space="PSUM") as ps_tr, \
             tc.tile_pool(name="ot_ps", bufs=1, space="PSUM") as ps_out:

            idf = cpool.tile([128, 128], F32)
            masks.make_identity(nc, idf[:])
            idb = cpool.tile([128, 128], BF16)
            masks.make_identity(nc, idb[:])
            ones_t = cpool.tile([1, 128], F32)
            nc.vector.memset(ones_t[:], 1.0)
            zrow = cpool.tile([1, 64], F32)
            nc.vector.memset(zrow[:], 0.0)
            bqkv_sb = cpool.tile([128, NT], F32)
            nc.sync.dma_start(bqkv_sb[:], bqkv_in.ap().rearrange("(t p) -> p t", p=128))
            bsc = cpool.tile([128, NT], F32)
            # q biases (tiles 0,1) pre-scaled by 1/sqrt(D)
            nc.scalar.mul(bsc[:, 0:2], bqkv_sb[:, 0:2], 1.0 / float(np.sqrt(D)))
            nc.scalar.copy(bsc[:, 2:NT], bqkv_sb[:, 2:NT])
            bo_sb = cpool.tile([1, E], F32)
            nc.sync.dma_start(bo_sb[:], bo_in.ap()[None, :])

            # ---- x -> x^T ----------------------------------------------------
            qkvT = [qpool.tile([128, S], F32, name=f"qkvT{t}", tag=f"qkvT{t}") for t in range(NT)]
            with tc.tile_pool(name="xload", bufs=3) as xpool, \
                 tc.tile_pool(name="xT", bufs=1) as xtp, \
                 tc.tile_pool(name="wq", bufs=1) as wpool:
                xT = [xtp.tile([128, S], F32, name=f"xT{k}", tag=f"xT{k}") for k in range(KT)]
                for m in range(ST):
                    xs = xpool.tile([128, E], F32, tag="xs")
                    nc.sync.dma_start(xs[:], x_in.ap()[m * 128:(m + 1) * 128, :])
                    for k in range(KT):
                        tp = ps_tr.tile([128, 512], F32, tag="tr")
                        nc.tensor.transpose(tp[:, 0:128], xs[:, k * 128:(k + 1) * 128],
                                            idf[:])
                        eng = nc.vector if (k % 2 == 0) else nc.scalar
                        if eng is nc.vector:
                            nc.vector.tensor_copy(xT[k][:, m * 128:(m + 1) * 128],
                                                  tp[:, 0:128])
                        else:
                            nc.scalar.copy(xT[k][:, m * 128:(m + 1) * 128],
                                           tp[:, 0:128])

                # ---- QKV projection (weights stationary) --------------------
                wsb = [wpool.tile([128, 3 * HPC * D], F32, name=f"w{k}", tag=f"w{k}")
                       for k in range(KT)]
                for k in range(KT):
                    nc.sync.dma_start(wsb[k][:], wqkv_in.ap()[k * 128:(k + 1) * 128, :])
                for t in range(NT):
                    scale = 1.0 / float(np.sqrt(D)) if t < 2 else 1.0
                    for sc in range(S // 512):
                        pt = ps_mm.tile([128, 512], F32, tag="mm")
                        for k in range(KT):
                            nc.tensor.matmul(pt[:], wsb[k][:, t * 128:(t + 1) * 128],
                                             xT[k][:, sc * 512:(sc + 1) * 512],
                                             start=(k == 0), stop=(k == KT - 1))
                        nc.scalar.activation(qkvT[t][:, sc * 512:(sc + 1) * 512], pt[:],
                                             AF.Identity, bias=bsc[:, t:t + 1],
                                             scale=scale)

            # ---- V^T -> V (normal layout, bf16) ------------------------------
            # Full-width transposes (both heads of a pair at once); all
            # transpose outputs at PSUM partition 0 (HW requirement).
            V = [vpool.tile([128, NB // 2 * D], BF16, name=f"V{h}", tag=f"V{h}") for h in range(HPC)]
            # odd j blocks that appear as singletons need a base-0 copy
            odd_singles = sorted({ent[1] for i in range(NB) for ent in entries[i]
                                  if ent[0] == 'single' and ent[1] % 2 == 1})
            odd_slot = {j: s for s, j in enumerate(odd_singles)}
            if odd_singles:
                Vodd = [vpool.tile([64, len(odd_singles) * D], BF16,
                                   name=f"Vodd{h}", tag=f"Vodd{h}")
                        for h in range(HPC)]
            else:
                Vodd = []
            for vp in range(2):                 # head pairs (0,1) and (2,3)
                vt = qkvT[4 + vp]
                for c4 in range(0, NB // 2, 4):  # 4 s-chunks per psum tile
                    tp = ps_tr.tile([128, 512], F32, tag="tr")
                    for u in range(4):
                        c = c4 + u
                        nc.tensor.transpose(tp[:, u * 128:(u + 1) * 128],
                                            vt[:, c * 128:(c + 1) * 128], idf[:])
                    for lh in range(2):
                        src = tp[:, 0:512].rearrange("p (n q) -> p n q", q=128)[
                            :, :, lh * 64:(lh + 1) * 64]
                        dst = V[2 * vp + lh][:, c4 * 64:(c4 + 4) * 64].rearrange(
                            "p (n q) -> p n q", q=64)
                        if lh == 0:
                            nc.scalar.copy(dst, src)
                        else:
                            nc.vector.tensor_copy(dst, src)
                for j4 in range(0, len(odd_singles), 4):
                    js = odd_singles[j4:j4 + 4]
                    tp = ps_tr.tile([128, 512], F32, tag="tr")
                    for u, j in enumerate(js):
                        nc.tensor.transpose(tp[0:64, u * 128:(u + 1) * 128],
                                            vt[:, j * 64:(j + 1) * 64], idf[:])
                    for lh in range(2):
                        src = tp[0:64, 0:len(js) * 128].rearrange(
                            "p (n q) -> p n q", q=128)[:, :, lh * 64:(lh + 1) * 64]
                        dst = Vodd[2 * vp + lh][:, j4 * 64:(j4 + len(js)) * 64] \
                            .rearrange("p (n q) -> p n q", q=64)
                        if lh == 0:
                            nc.scalar.copy(dst, src)
                        else:
                            nc.vector.tensor_copy(dst, src)

            outSB = [opool.tile([128, S], F32, name=f"outSB{hp}", tag=f"outSB{hp}") for hp in range(2)]

            if stage == 'proj':
                prb = opool.tile([128, E], F32, name="prb")
                nc.vector.tensor_copy(prb[:], qkvT[0][:, 0:E])
                nc.sync.dma_start(y_out.ap()[0:128, :], prb[:])
                nc.vector.tensor_copy(prb[:], V[3][:, 0:E].bitcast(F32).broadcast_to([128, E]) if False else qkvT[5][:, 0:E])
                nc.sync.dma_start(y_out.ap()[128:256, :], prb[:])

            # ---- attention per head pair ------------------------------------
            with tc.tile_pool(name="attnT", bufs=1) as apool:
             if stage != 'proj':
              for hp in range(2):
                ha, hb_ = 2 * hp, 2 * hp + 1
                qT = qkvT[hp]          # [128, S] heads (ha at 0:64, hb at 64:128)
                kT = qkvT[2 + hp]
                aT = [apool.tile([128, ncols_total * 64], BF16, name=f"aT{hp}_{h}", tag=f"aT{h}")
                      for h in range(2)]

                with tc.tile_pool(name=f"exp{hp}", bufs=3) as epool, \
                     tc.tile_pool(name=f"att{hp}", bufs=3) as atpool, \
                     tc.tile_pool(name=f"den{hp}", bufs=4) as dpool:
                    # -------- Phase A --------
                    for i in range(NB):
                        for sc in score_chunks[i]:
                            jlist = sc['jlist']
                            nbk = len(jlist)
                            spt = ps_mm.tile([128, 512], F32, tag="mm")
                            # QK^T for both heads, runs of consecutive j
                            for lh, pb in ((0, 0), (1, 64)):
                                mm_list = []
                                pos = 0
                                for (j0, rl) in _runs(jlist):
                                    mm_list.append((pos, j0, rl))
                                    pos += rl
                                for mi, (pos, j0, rl) in enumerate(mm_list):
                                    nc.tensor.matmul(
                                        spt[pb:pb + 64, pos * 64:(pos + rl) * 64],
                                        qT[pb:pb + 64, i * 64:(i + 1) * 64],
                                        kT[pb:pb + 64, j0 * 64:(j0 + rl) * 64],
                                        start=(mi == 0), stop=(mi == len(mm_list) - 1),
                                        tile_position=(pb, pb))
                            ex = epool.tile([128, 512], F32, tag="exp")
                            nc.scalar.activation(ex[:, 0:nbk * 64], spt[:, 0:nbk * 64],
                                                 AF.Exp)
                            den = dpool.tile([128, 8], F32, tag="den")
                            rec = dpool.tile([128, 8], F32, tag="rec")
                            nc.vector.tensor_reduce(
                                den[:, 0:nbk],
                                ex[:, 0:nbk * 64].rearrange("p (n k) -> p n k", k=64),
                                axis=AX.X, op=ALU.add)
                            nc.vector.reciprocal(rec[:, 0:nbk], den[:, 0:nbk])
                            att = atpool.tile([128, 512], BF16, tag="att")
                            nc.gpsimd.tensor_tensor(
                                att[:, 0:nbk * 64].rearrange("p (n k) -> p n k", k=64),
                                ex[:, 0:nbk * 64].rearrange("p (n k) -> p n k", k=64),
                                rec[:, 0:nbk, None].to_broadcast((128, nbk, 64)),
                                ALU.mult)
                            # -------- transposes into attnT storage --------
                            jpos = {j: p for p, j in enumerate(jlist)}
                            groups = [('pair', sc['pairs'], 128, 0),
                                      ('sing', sc['sE'] + sc['sO'], 64, 0)]
                            e_run = sc['e0']   # entry index walker (storage order)
                            for kind, glist, prange, pbase in groups:
                                for g0 in range(0, len(glist), 8):
                                    gl = glist[g0:g0 + 8]
                                    if not gl:
                                        continue
                                    n = len(gl)
                                    tp = ps_tr.tile([128, 1024], BF16, tag="tr")
                                    for u, ent in enumerate(gl):
                                        jj = ent[1]
                                        sp = jpos[jj]
                                        if kind == 'pair':
                                            nc.tensor.transpose(
                                                tp[:, u * 128:(u + 1) * 128],
                                                att[:, sp * 64:(sp + 2) * 64],
                                                idb[:])
                                        else:
                                            nc.tensor.transpose(
                                                tp[0:64, u * 128:(u + 1) * 128],
                                                att[:, sp * 64:(sp + 1) * 64],
                                                idb[:])
                                    # copies to attnT (one instr per head)
                                    c0 = col[i][e_run]
                                    for lh in range(2):
                                        src_ap = tp[pbase:pbase + prange,
                                                    0:n * 128].rearrange(
                                            "p (n q) -> p n q", q=128)[:, :, lh * 64:(lh + 1) * 64]
                                        dst_ap = aT[lh][pbase:pbase + prange,
                                                        c0 * 64:(c0 + n) * 64].rearrange(
                                            "p (n q) -> p n q", q=64)
                                        if lh == 0:
                                            nc.scalar.copy(dst_ap, src_ap)
                                        else:
                                            nc.vector.tensor_copy(dst_ap, src_ap)
                                    e_run += n

                    # -------- Phase B --------
                    if stage == 'phaseA':
                        prb2 = epool.tile([128, 512], F32, tag="prbA")
                        nc.vector.tensor_copy(prb2[:], aT[0][:, 0:1024].bitcast(F32))
                        nc.sync.dma_start(
                            y_out.ap()[hp * 128:(hp + 1) * 128, 0:512], prb2[:])
                        continue
                    otp = ps_out.tile([128, S], F32, tag="ot")
                    for lh, ob in ((0, 0), (1, 64)):
                        h = 2 * hp + lh
                        # build MM descriptors: (c_or_None, j_or_None, i, e_idx)
                        mms = []
                        for i in range(NB):
                            if not entries[i]:
                                mms.append(('zero', None, i, None))
                        by_c = {}
                        for i in range(NB):
                            for e_idx, ent in enumerate(entries[i]):
                                kind, j = ent
                                c = j // 2
                                by_c.setdefault(c, []).append((kind, j, i, e_idx))
                        for c in sorted(by_c):
                            for kind, j, i, e_idx in by_c[c]:
                                mms.append((kind, j, i, e_idx))
                        # bank first/last bookkeeping (bank = i//8)
                        first_mm, last_mm = {}, {}
                        for mi, (kind, j, i, e_idx) in enumerate(mms):
                            bk = i // 8
                            if bk not in first_mm:
                                first_mm[bk] = mi
                            last_mm[bk] = mi
                        for mi, (kind, j, i, e_idx) in enumerate(mms):
                            bk = i // 8
                            st = (first_mm[bk] == mi)
                            sp = (last_mm[bk] == mi)
                            oap = otp[ob:ob + 64, i * 64:(i + 1) * 64]
                            if kind == 'zero':
                                nc.tensor.matmul(oap, ones_t[:, 0:64], zrow[:],
                                                 start=st, stop=sp,
                                                 tile_position=(0, ob))
                                continue
                            cc = j // 2
                            colw = col[i][e_idx]
                            if kind == 'pair':
                                nc.tensor.matmul(
                                    oap,
                                    V[h][:, cc * 64:(cc + 1) * 64],
                                    aT[lh][:, colw * 64:(colw + 1) * 64],
                                    start=st, stop=sp, tile_position=(0, ob))
                            elif j % 2 == 0:
                                nc.tensor.matmul(
                                    oap,
                                    V[h][0:64, cc * 64:(cc + 1) * 64],
                                    aT[lh][0:64, colw * 64:(colw + 1) * 64],
                                    start=st, stop=sp, tile_position=(0, ob))
                            else:
                                slot = odd_slot[j]
                                nc.tensor.matmul(
                                    oap,
                                    Vodd[h][0:64, slot * 64:(slot + 1) * 64],
                                    aT[lh][0:64, colw * 64:(colw + 1) * 64],
                                    start=st, stop=sp, tile_position=(0, ob))
                    for sc4 in range(4):
                        nc.scalar.copy(outSB[hp][:, sc4 * 512:(sc4 + 1) * 512],
                                       otp[:, sc4 * 512:(sc4 + 1) * 512])

            # ---- Wo projection ----------------------------------------------
            if stage in ('phaseA', 'phaseB'):
                if stage == 'phaseB':
                    with tc.tile_pool(name="prbB", bufs=1) as pbp:
                        prb3 = pbp.tile([128, S], F32, name="prb3")
                        nc.vector.tensor_copy(prb3[:], outSB[0][:])
                        nc.sync.dma_start(
                            y_out.ap()[0:128, :], prb3[:, 0:E])
                nc.compile()
                return nc
            with tc.tile_pool(name="wo", bufs=1) as wop, \
                 tc.tile_pool(name="yout", bufs=3) as ypool:
                wo_sb = [wop.tile([128, E], F32, name=f"wo{hp}", tag=f"wo{hp}") for hp in range(2)]
                for hp in range(2):
                    nc.sync.dma_start(wo_sb[hp][:],
                                      wo_in.ap()[hp * 128:(hp + 1) * 128, :])
                for st_ in range(ST):
                    yt = ypool.tile([128, E], F32, tag="yt")
                    for nchk in range(2):
                        pt = ps_mm.tile([128, 512], F32, tag="mm")
                        for hp in range(2):
                            nc.tensor.matmul(pt[:],
                                             outSB[hp][:, st_ * 128:(st_ + 1) * 128],
                                             wo_sb[hp][:, nchk * 512:(nchk + 1) * 512],
                                             start=(hp == 0), stop=False)
                        nc.tensor.matmul(pt[:], ones_t[:],
                                         bo_sb[:, nchk * 512:(nchk + 1) * 512],
                                         start=False, stop=True)
                        nc.scalar.copy(yt[:, nchk * 512:(nchk + 1) * 512], pt[:])
                    nc.sync.dma_start(y_out.ap()[st_ * 128:(st_ + 1) * 128, :], yt[:])

    nc.compile()
    return nc


# ---------------------------------------------------------------- entry point

def kernel(x, Wq, bq, Wk, bk, Wv, bv, Wo, bo, block_rows, block_cols):
    global LAST_RESULTS
    from concourse.bass_utils import run_bass_kernel_spmd
    import os

    x = np.asarray(x, dtype=np.float32)
    Wq, Wk, Wv, Wo = (np.asarray(a, dtype=np.float32) for a in (Wq, Wk, Wv, Wo))
    bq, bk, bv, bo = (np.asarray(a, dtype=np.float32) for a in (bq, bk, bv, bo))

    plan = _plan(block_rows, block_cols)
    nc = _build_program(plan)

    in_maps = []
    for c in range(NCORES):
        b, g = c // 4, c % 4
        cs = slice(g * HPC * D, (g + 1) * HPC * D)
        w_qkv = np.ascontiguousarray(
            np.concatenate([Wq[:, cs], Wk[:, cs], Wv[:, cs]], axis=1))
        b_qkv = np.ascontiguousarray(
            np.concatenate([bq[cs], bk[cs], bv[cs]]))
        w_o = np.ascontiguousarray(Wo[cs, :])
        b_o = bo if g == 0 else np.zeros_like(bo)
        in_maps.append(dict(x_local=np.ascontiguousarray(x[b]),
                            w_qkv=w_qkv, b_qkv=b_qkv, w_o=w_o,
                            b_o=np.ascontiguousarray(b_o)))

    trace = bool(int(os.environ.get("KERNEL_TRACE", "0")))
    res = run_bass_kernel_spmd(nc, in_maps, core_ids=list(range(NCORES)),
                               trace=trace)
    LAST_RESULTS = res

    y = np.zeros((B, S, E), dtype=np.float32)
    for c in range(NCORES):
        y[c // 4] += res.results[c]["y_partial"]
    return y



# revision 8
# speedup vs baseline: 1.2566x; 1.2566x over previous
"""Block-sparse attention Trainium2 kernel (8 NeuronCores, SPMD).

Sharding: data-parallel over (batch, head-group): core c handles batch b=c//4
and heads [4*(c%4) .. 4*(c%4)+4). Block index lists are replicated (used
host-side to build the static program). Each core returns a partial
[S, E] output (its heads' contribution through Wo); the host sums the 4
partials per batch and adds bo once.

Design (transposed-scores dataflow, j-major):
  Host feeds x^T; QKV projection runs weights-stationary in float32r
  (1 cyc/row at 512-wide moving) producing q^T,k^T (bf16, q pre-scaled
  by 1/sqrt(D)) and v^T (bf16). V is put in [keys, d] layout by PE
  transposes. Per head pair, block-diagonal stationaries are built by
  SBUF->SBUF DMA:
    kdiag_j = [[kT_A(j), 0], [0, kT_B(j)]]   (dA|dB x keysA|keysB)
    Vdiag_j = [[V_A(j), 0], [0, V_B(j)]]     (keysA|keysB x dA|dB)
  For each col-block j and chunk of row-blocks i (<=8 blocks, 512 cols):
    scoresT = kdiag_j^T @ qT[:, i-cols]   -> PSUM [128=keysA|keysB, cols]
    expT    = ACT exp -> SBUF bf16
    denB    = onesdiag^T @ expT           -> per-head key-sums replicated
                                             across that head's 64 partitions
    att2T   = expT / denB                 (DVE divide, elementwise)
    out^T  += Vdiag_j^T @ att2T           -> PSUM otp2 [128=dA|dB, S]
  No attn transposes, no reduce, no recip, no attnT copies.
  Wo projection in float32r from out^T tiles; bias added on host.
"""
import numpy as np

B, S, E, H, D, BS = 2, 2048, 1024, 16, 64, 64
NB = S // BS          # 32
NCORES = 8
HPC = 4               # heads per core

LAST_RESULTS = None   # BassKernelResults of the most recent run (for test.py)

# chunk indices (global, per head-pair) whose divide runs on gpsimd instead
# of DVE; tune for engine balance.
DIV_GPSIMD_MOD = 0    # 0 = all on DVE; k>0 = every k-th chunk on gpsimd


# ---------------------------------------------------------------- host planning

def _plan(block_rows, block_cols):
    """j-major static schedule shared by every head-pair/core.

    For each col-block j: active row-blocks i, split into maximal
    consecutive runs that do not cross multiples of 8 (PSUM bank alignment
    for the out^T accumulator), grouped into chunks of <=8 blocks
    (<=512 cols, one PSUM bank per scores/den tile).

    Returns:
      chunks: list of (j, [(pos, i0, n), ...]) in emission order; pos is the
              64-col block offset inside the chunk tile.
      flags:  dict (j, i0) -> [start, stop] for the otp2 accumulation.
    """
    mask = np.zeros((NB, NB), dtype=bool)
    for r, c in zip(np.asarray(block_rows).tolist(), np.asarray(block_cols).tolist()):
        mask[int(r), int(c)] = True

    chunks = []
    pieces_by_bank = {}
    for j in range(NB):
        ilist = np.nonzero(mask[:, j])[0].tolist()
        runs = []
        cur = None
        for i in ilist:
            if cur is not None and i == cur[0] + cur[1] and (i % 8 != 0):
                cur[1] += 1
            else:
                cur = [i, 1]
                runs.append(cur)
        # group runs into chunks of <=8 blocks
        group, nb_ = [], 0
        for i0, n in runs:
            if nb_ + n > 8:
                chunks.append((j, group))
                group, nb_ = [], 0
            group.append((nb_, i0, n))
            nb_ += n
        if group:
            chunks.append((j, group))
    # otp2 start/stop: first/last piece per 8-i bank in emission order
    order = []
    for j, group in chunks:
        for pos, i0, n in group:
            order.append((j, i0, n))
    flags = {}
    first_seen, last_seen = {}, {}
    for idx, (j, i0, n) in enumerate(order):
        bk = i0 // 8
        assert (i0 + n - 1) // 8 == bk
        if bk not in first_seen:
            first_seen[bk] = (j, i0)
        last_seen[bk] = (j, i0)
        flags[(j, i0)] = [False, False]
    for bk, key in first_seen.items():
        flags[key][0] = True
    for bk, key in last_seen.items():
        flags[key][1] = True
    return dict(chunks=chunks, flags=flags)


# ---------------------------------------------------------------- bass program

def _build_program(plan):
    import concourse.bacc as bacc
    import concourse.mybir as mybir
    from concourse.tile import TileContext
    from concourse import masks

    F32 = mybir.dt.float32
    F32R = mybir.dt.float32r
    BF16 = mybir.dt.bfloat16
    AF = mybir.ActivationFunctionType
    ALU = mybir.AluOpType

    nc = bacc.Bacc("TRN2", target_bir_lowering=False, debug=False)

    xT_in = nc.dram_tensor("xT_local", [E, S], F32R, kind="ExternalInput")
    wqkv_in = nc.dram_tensor("w_qkv", [E, 3 * HPC * D], F32R, kind="ExternalInput")
    bqkv_in = nc.dram_tensor("b_qkv", [3 * HPC * D], F32, kind="ExternalInput")
    wo_in = nc.dram_tensor("w_o", [HPC * D, E], F32R, kind="ExternalInput")
    y_out = nc.dram_tensor("y_partial", [S, E], F32, kind="ExternalOutput")

    NT = 3 * HPC * D // 128      # 6 qkv n-tiles
    KT = E // 128                # 8 contraction tiles
    ST = S // 128                # 16 s tiles
    SC = S // 512                # 4 s-chunks

    chunks, flags = plan['chunks'], plan['flags']

    with TileContext(nc) as tc:
        with tc.tile_pool(name="const", bufs=1) as cpool, \
             tc.tile_pool(name="qk", bufs=1) as qkpool, \
             tc.tile_pool(name="vt", bufs=1) as vtpool, \
             tc.tile_pool(name="diag", bufs=1) as dgpool, \
             tc.tile_pool(name="outsb", bufs=1) as opool, \
             tc.tile_pool(name="wo", bufs=1) as wop:

            idb = cpool.tile([128, 128], BF16)
            masks.make_identity(nc, idb[:])
            bqkv_sb = cpool.tile([128, NT], F32)
            nc.sync.dma_start(bqkv_sb[:], bqkv_in.ap().rearrange("(t p) -> p t", p=128))
            bsc = cpool.tile([128, NT], F32)
            nc.scalar.mul(bsc[:, 0:2], bqkv_sb[:, 0:2], 0.125)
            nc.scalar.copy(bsc[:, 2:NT], bqkv_sb[:, 2:NT])
            onesdiag = cpool.tile([128, 128], BF16)
            nc.gpsimd.memset(onesdiag[:], 0.0)
            nc.gpsimd.memset(onesdiag[0:64, 0:64], 1.0)
            nc.gpsimd.memset(onesdiag[64:128, 64:128], 1.0)

            wo_sb = [wop.tile([128, E], F32R, name=f"wo{hp}") for hp in range(2)]
            for hp in range(2):
                nc.sync.dma_start(wo_sb[hp][:],
                                  wo_in.ap()[hp * 128:(hp + 1) * 128, :])

            # q0,q1,k0,k1 bf16 [128, S]; vT per pair bf16
            qT = [qkpool.tile([128, S], BF16, name=f"qT{hp}") for hp in range(2)]
            kT = [qkpool.tile([128, S], BF16, name=f"kT{hp}") for hp in range(2)]
            vT = [vtpool.tile([128, S], BF16, name=f"vT{hp}") for hp in range(2)]
            V = [vtpool.tile([128, (NB // 2) * D], BF16, name=f"V{h}") for h in range(HPC)]
            kdiag = [dgpool.tile([128, NB * 128], BF16, name=f"kdiag{hp}") for hp in range(2)]
            Vdiag = [dgpool.tile([128, NB * 128], BF16, name=f"Vdiag{hp}") for hp in range(2)]
            for hp in range(2):
                nc.gpsimd.memset(kdiag[hp][:], 0.0)
                nc.gpsimd.memset(Vdiag[hp][:], 0.0)
            outSB = [opool.tile([128, S], F32R, name=f"outSB{hp}") for hp in range(2)]

            # ---- QKV projection (weights stationary, f32r) -----------------
            qkv_dst = [qT[0], qT[1], kT[0], kT[1], vT[0], vT[1]]
            with tc.tile_pool(name="xin", bufs=2) as xpool, \
                 tc.tile_pool(name="wq", bufs=1) as wpool, \
                 tc.tile_pool(name="qkv_ps", bufs=4, space="PSUM") as qkv_ps, \
                 tc.tile_pool(name="tr_ps", bufs=2, space="PSUM") as tr_ps:
                wsb = [wpool.tile([128, 3 * HPC * D], F32R, name=f"w{k}")
                       for k in range(KT)]
                for k in range(KT):
                    nc.sync.dma_start(wsb[k][:], wqkv_in.ap()[k * 128:(k + 1) * 128, :])
                xT_v = xT_in.ap().rearrange("(k p) s -> p k s", p=128)
                for sc in range(SC):
                    xsc = xpool.tile([128, KT, 512], F32R, tag="xsc")
                    nc.sync.dma_start(xsc[:], xT_v[:, :, sc * 512:(sc + 1) * 512])
                    for t in range(NT):
                        pt = qkv_ps.tile([128, 512], F32, tag="qkvmm")
                        for k in range(KT):
                            nc.tensor.matmul(
                                pt[:],
                                wsb[k][:, t * 128:(t + 1) * 128],
                                xsc[:, k, :],
                                start=(k == 0), stop=(k == KT - 1))
                        nc.scalar.activation(
                            qkv_dst[t][:, sc * 512:(sc + 1) * 512], pt[:],
                            AF.Identity, bias=bsc[:, t:t + 1],
                            scale=0.125 if t < 2 else 1.0)

                # ---- V: vT -> V[h] ([keys, d] layout) ----------------------
                for vp in range(2):
                    for c4 in range(0, NB // 2, 4):
                        tp = tr_ps.tile([128, 512], BF16, tag="vtr")
                        for u in range(4):
                            c = c4 + u
                            nc.tensor.transpose(tp[:, u * 128:(u + 1) * 128],
                                                vT[vp][:, c * 128:(c + 1) * 128],
                                                idb[:])
                        for lh in range(2):
                            src = tp[:, 0:512].rearrange("p (u x) -> p u x", x=128)[
                                :, :, lh * 64:(lh + 1) * 64]
                            dst = V[2 * vp + lh][:, c4 * 64:(c4 + 4) * 64].rearrange(
                                "p (u d) -> p u d", d=64)
                            if lh == 0:
                                nc.scalar.copy(dst, src)
                            else:
                                nc.vector.tensor_copy(dst, src)

            # ---- block-diagonal stationaries via SBUF->SBUF DMA ------------
            for hp in range(2):
                kd = kdiag[hp][:, :]
                for lh, (p0, c0) in enumerate(((0, 0), (64, 64))):
                    dst = kd[p0:p0 + 64, :].rearrange("p (j c) -> p j c", c=128)[
                        :, :, c0:c0 + 64]
                    src = kT[hp][p0:p0 + 64, :].rearrange("p (j c) -> p j c", c=64)
                    nc.sync.dma_start(dst, src)
                vd = Vdiag[hp][:, :]
                for lh in range(2):
                    h = 2 * hp + lh
                    pd, cd = (0, 0) if lh == 0 else (64, 64)
                    for par in range(2):   # even j at V parts 0:64, odd at 64:128
                        dst = vd[pd:pd + 64, :].rearrange(
                            "p (c x) -> p c x", x=256)[:, :, par * 128 + cd:par * 128 + cd + 64]
                        src = V[h][par * 64:(par + 1) * 64, :].rearrange(
                            "p (c d) -> p c d", d=64)
                        nc.sync.dma_start(dst, src)

            # ---- attention, j-major, per head pair -------------------------
            with tc.tile_pool(name="sc_ps", bufs=2, space="PSUM") as sc_ps, \
                 tc.tile_pool(name="dn_ps", bufs=2, space="PSUM") as dn_ps, \
                 tc.tile_pool(name="ot_ps", bufs=1, space="PSUM") as ot_ps, \
                 tc.tile_pool(name="ex", bufs=4) as expool, \
                 tc.tile_pool(name="rc", bufs=4) as rcpool, \
                 tc.tile_pool(name="at", bufs=4) as atpool:
                for hp in range(2):
                    otp2 = ot_ps.tile([128, S], F32, tag="otp")
                    for ci, (j, group) in enumerate(chunks):
                        ncols = sum(n for _, _, n in group) * 64
                        spt = sc_ps.tile([128, 512], F32, tag="spt")
                        for gi, (pos, i0, n) in enumerate(group):
                            nc.tensor.matmul(
                                spt[:, pos * 64:(pos + n) * 64],
                                kdiag[hp][:, j * 128:(j + 1) * 128],
                                qT[hp][:, i0 * 64:(i0 + n) * 64],
                                start=(gi == 0), stop=(gi == len(group) - 1))
                        ex = expool.tile([128, 512], BF16, tag="ex")
                        nc.scalar.activation(ex[:, 0:ncols], spt[:, 0:ncols], AF.Exp)
                        dnb = dn_ps.tile([128, 512], F32, tag="dnb")
                        nc.tensor.matmul(dnb[:, 0:ncols], onesdiag[:],
                                         ex[:, 0:ncols], start=True, stop=True)
                        rec = rcpool.tile([128, 512], BF16, tag="rec")
                        with nc.allow_low_precision(reason="softmax recip in bf16"):
                            nc.vector.reciprocal(rec[:, 0:ncols], dnb[:, 0:ncols])
                        at2 = atpool.tile([128, 512], BF16, tag="at2")
                        if ci % 3 == 2:
                            nc.vector.tensor_tensor(at2[:, 0:ncols], ex[:, 0:ncols],
                                                    rec[:, 0:ncols], ALU.mult)
                        else:
                            nc.gpsimd.tensor_tensor(at2[:, 0:ncols], ex[:, 0:ncols],
                                                    rec[:, 0:ncols], ALU.mult)
                        for pos, i0, n in group:
                            st, sp = flags[(j, i0)]
                            nc.tensor.matmul(
                                otp2[:, i0 * 64:(i0 + n) * 64],
                                Vdiag[hp][:, j * 128:(j + 1) * 128],
                                at2[:, pos * 64:(pos + n) * 64],
                                start=st, stop=sp)
                    for q in range(4):
                        if q % 2 == 0:
                            nc.scalar.copy(outSB[hp][:, q * 512:(q + 1) * 512],
                                           otp2[:, q * 512:(q + 1) * 512])
                        else:
                            nc.vector.tensor_copy(outSB[hp][:, q * 512:(q + 1) * 512],
                                                  otp2[:, q * 512:(q + 1) * 512])

            # ---- Wo projection (f32r), bias added on host ------------------
            with tc.tile_pool(name="yt", bufs=3) as ypool, \
                 tc.tile_pool(name="wo_ps", bufs=2, space="PSUM") as wo_ps:
                for st_ in range(ST):
                    yt = ypool.tile([128, E], F32, tag="yt")
                    for nchk in range(2):
                        pt = wo_ps.tile([128, 512], F32, tag="womm")
                        for hp in range(2):
                            nc.tensor.matmul(
                                pt[:],
                                outSB[hp][:, st_ * 128:(st_ + 1) * 128],
                                wo_sb[hp][:, nchk * 512:(nchk + 1) * 512],
                                start=(hp == 0), stop=(hp == 1))
                        if nchk == 0:
                            nc.scalar.copy(yt[:, nchk * 512:(nchk + 1) * 512], pt[:])
                        else:
                            nc.vector.tensor_copy(yt[:, nchk * 512:(nchk + 1) * 512],
                                                  pt[:])
                    nc.sync.dma_start(y_out.ap()[st_ * 128:(st_ + 1) * 128, :], yt[:])

    nc.compile()
    return nc


# ---------------------------------------------------------------- entry point

def kernel(x, Wq, bq, Wk, bk, Wv, bv, Wo, bo, block_rows, block_cols):
    global LAST_RESULTS
    from concourse.bass_utils import run_bass_kernel_spmd
    import os

    x = np.asarray(x, dtype=np.float32)
    Wq, Wk, Wv, Wo = (np.asarray(a, dtype=np.float32) for a in (Wq, Wk, Wv, Wo))
    bq, bk, bv, bo = (np.asarray(a, dtype=np.float32) for a in (bq, bk, bv, bo))

    plan = _plan(block_rows, block_cols)
    nc = _build_program(plan)

    xT = [np.ascontiguousarray(x[b].T) for b in range(B)]
    in_maps = []
    for c in range(NCORES):
        b, g = c // 4, c % 4
        cs = slice(g * HPC * D, (g + 1) * HPC * D)
        w_qkv = np.ascontiguousarray(
            np.concatenate([Wq[:, cs], Wk[:, cs], Wv[:, cs]], axis=1))
        b_qkv = np.ascontiguousarray(
            np.concatenate([bq[cs], bk[cs], bv[cs]]))
        w_o = np.ascontiguousarray(Wo[cs, :])
        in_maps.append(dict(xT_local=xT[b], w_qkv=w_qkv, b_qkv=b_qkv, w_o=w_o))

    trace = bool(int(os.environ.get("KERNEL_TRACE", "0")))
    res = run_bass_kernel_spmd(nc, in_maps, core_ids=list(range(NCORES)),
                               trace=trace)
    LAST_RESULTS = res

    y = np.zeros((B, S, E), dtype=np.float32)
    for c in range(NCORES):
        y[c // 4] += res.results[c]["y_partial"]
    y += bo
    return y


# revision 9
# speedup vs baseline: 2.6172x; 2.0828x over previous
"""Block-sparse attention Trainium2 kernel (8 NeuronCores, SPMD).

Sharding: data-parallel over (batch, head-group): core c handles batch b=c//4
and heads [4*(c%4) .. 4*(c%4)+4). Block index lists are replicated (used
host-side to build the static program). Each core returns a partial
[S, E] output (its heads' contribution through Wo); the host sums the 4
partials per batch and adds bo once.

Design (transposed-scores dataflow, j-major):
  Host feeds x^T; QKV projection runs weights-stationary in float32r
  (1 cyc/row at 512-wide moving) producing q^T,k^T (bf16, q pre-scaled
  by 1/sqrt(D)) and v^T (bf16). V is put in [keys, d] layout by PE
  transposes. Per head pair, block-diagonal stationaries are built by
  SBUF->SBUF DMA:
    kdiag_j = [[kT_A(j), 0], [0, kT_B(j)]]   (dA|dB x keysA|keysB)
    Vdiag_j = [[V_A(j), 0], [0, V_B(j)]]     (keysA|keysB x dA|dB)
  For each col-block j and chunk of row-blocks i (<=8 blocks, 512 cols):
    scoresT = kdiag_j^T @ qT[:, i-cols]   -> PSUM [128=keysA|keysB, cols]
    expT    = ACT exp -> SBUF bf16
    denB    = onesdiag^T @ expT           -> per-head key-sums replicated
                                             across that head's 64 partitions
    att2T   = expT / denB                 (DVE divide, elementwise)
    out^T  += Vdiag_j^T @ att2T           -> PSUM otp2 [128=dA|dB, S]
  No attn transposes, no reduce, no recip, no attnT copies.
  Wo projection in float32r from out^T tiles; bias added on host.
"""
import numpy as np

B, S, E, H, D, BS = 2, 2048, 1024, 16, 64, 64
NB = S // BS          # 32
NCORES = 8
HPC = 4               # heads per core

LAST_RESULTS = None   # BassKernelResults of the most recent run (for test.py)

# chunk indices (global, per head-pair) whose divide runs on gpsimd instead
# of DVE; tune for engine balance.
DIV_GPSIMD_MOD = 0    # 0 = all on DVE; k>0 = every k-th chunk on gpsimd


# ---------------------------------------------------------------- host planning

def _plan(block_rows, block_cols):
    """j-major static schedule shared by every head-pair/core.

    For each col-block j: active row-blocks i, split into maximal
    consecutive runs that do not cross multiples of 8 (PSUM bank alignment
    for the out^T accumulator), grouped into chunks of <=8 blocks
    (<=512 cols, one PSUM bank per scores/den tile).

    Returns:
      chunks: list of (j, [(pos, i0, n), ...]) in emission order; pos is the
              64-col block offset inside the chunk tile.
      flags:  dict (j, i0) -> [start, stop] for the otp2 accumulation.
    """
    mask = np.zeros((NB, NB), dtype=bool)
    for r, c in zip(np.asarray(block_rows).tolist(), np.asarray(block_cols).tolist()):
        mask[int(r), int(c)] = True

    chunks = []
    pieces_by_bank = {}
    for j in range(NB):
        ilist = np.nonzero(mask[:, j])[0].tolist()
        runs = []
        cur = None
        for i in ilist:
            if cur is not None and i == cur[0] + cur[1] and (i % 8 != 0):
                cur[1] += 1
            else:
                cur = [i, 1]
                runs.append(cur)
        # group runs into chunks of <=8 blocks
        group, nb_ = [], 0
        for i0, n in runs:
            if nb_ + n > 8:
                chunks.append((j, group))
                group, nb_ = [], 0
            group.append((nb_, i0, n))
            nb_ += n
        if group:
            chunks.append((j, group))
    # otp2 start/stop: first/last piece per 8-i bank in emission order
    order = []
    for j, group in chunks:
        for pos, i0, n in group:
            order.append((j, i0, n))
    flags = {}
    first_seen, last_seen = {}, {}
    for idx, (j, i0, n) in enumerate(order):
        bk = i0 // 8
        assert (i0 + n - 1) // 8 == bk
        if bk not in first_seen:
            first_seen[bk] = (j, i0)
        last_seen[bk] = (j, i0)
        flags[(j, i0)] = [False, False]
    for bk, key in first_seen.items():
        flags[key][0] = True
    for bk, key in last_seen.items():
        flags[key][1] = True
    return dict(chunks=chunks, flags=flags)


# ---------------------------------------------------------------- bass program

def _build_program(plan):
    import concourse.bacc as bacc
    import concourse.mybir as mybir
    from concourse.tile import TileContext
    from concourse import masks

    F32 = mybir.dt.float32
    F32R = mybir.dt.float32r
    BF16 = mybir.dt.bfloat16
    AF = mybir.ActivationFunctionType
    ALU = mybir.AluOpType

    nc = bacc.Bacc("TRN2", target_bir_lowering=False, debug=False)

    xT_in = nc.dram_tensor("xT_local", [E, S], F32R, kind="ExternalInput")
    wqkv_in = nc.dram_tensor("w_qkv", [E, 3 * HPC * D], F32R, kind="ExternalInput")
    bqkv_in = nc.dram_tensor("b_qkv", [3 * HPC * D], F32, kind="ExternalInput")
    wo_in = nc.dram_tensor("w_o", [HPC * D, E], F32R, kind="ExternalInput")
    y_out = nc.dram_tensor("y_partial", [S, E], F32, kind="ExternalOutput")

    NT = 3 * HPC * D // 128      # 6 qkv n-tiles
    KT = E // 128                # 8 contraction tiles
    ST = S // 128                # 16 s tiles
    SC = S // 512                # 4 s-chunks

    chunks, flags = plan['chunks'], plan['flags']

    with TileContext(nc) as tc:
        with tc.tile_pool(name="const", bufs=1) as cpool, \
             tc.tile_pool(name="qk", bufs=1) as qkpool, \
             tc.tile_pool(name="vt", bufs=1) as vtpool, \
             tc.tile_pool(name="diag", bufs=1) as dgpool, \
             tc.tile_pool(name="outsb", bufs=1) as opool, \
             tc.tile_pool(name="wo", bufs=1) as wop:

            idb = cpool.tile([128, 128], BF16)
            masks.make_identity(nc, idb[:])
            bqkv_sb = cpool.tile([128, NT], F32)
            nc.sync.dma_start(bqkv_sb[:], bqkv_in.ap().rearrange("(t p) -> p t", p=128))
            bsc = cpool.tile([128, NT], F32)
            nc.scalar.mul(bsc[:, 0:2], bqkv_sb[:, 0:2], 0.125)
            nc.scalar.copy(bsc[:, 2:NT], bqkv_sb[:, 2:NT])
            onesdiag = cpool.tile([128, 128], BF16)
            nc.gpsimd.memset(onesdiag[:], 0.0)
            nc.gpsimd.memset(onesdiag[0:64, 0:64], 1.0)
            nc.gpsimd.memset(onesdiag[64:128, 64:128], 1.0)

            wo_sb = [wop.tile([128, E], F32R, name=f"wo{hp}") for hp in range(2)]
            for hp in range(2):
                nc.sync.dma_start(wo_sb[hp][:],
                                  wo_in.ap()[hp * 128:(hp + 1) * 128, :])

            # q0,q1,k0,k1 bf16 [128, S]; vT per pair bf16
            qT = [qkpool.tile([128, S], BF16, name=f"qT{hp}") for hp in range(2)]
            kT = [qkpool.tile([128, S], BF16, name=f"kT{hp}") for hp in range(2)]
            vT = [vtpool.tile([128, S], BF16, name=f"vT{hp}") for hp in range(2)]
            V = [vtpool.tile([128, (NB // 2) * D], BF16, name=f"V{h}") for h in range(HPC)]
            kdiag = [dgpool.tile([128, NB * 128], BF16, name=f"kdiag{hp}") for hp in range(2)]
            Vdiag = [dgpool.tile([128, NB * 128], BF16, name=f"Vdiag{hp}") for hp in range(2)]
            for hp in range(2):
                nc.gpsimd.memset(kdiag[hp][:], 0.0)
                nc.gpsimd.memset(Vdiag[hp][:], 0.0)
            outSB = [opool.tile([128, S], F32R, name=f"outSB{hp}") for hp in range(2)]

            # ---- QKV projection (weights stationary, f32r) -----------------
            qkv_dst = [qT[0], qT[1], kT[0], kT[1], vT[0], vT[1]]
            with tc.tile_pool(name="xin", bufs=2) as xpool, \
                 tc.tile_pool(name="wq", bufs=1) as wpool, \
                 tc.tile_pool(name="qkv_ps", bufs=4, space="PSUM") as qkv_ps, \
                 tc.tile_pool(name="tr_ps", bufs=2, space="PSUM") as tr_ps:
                wsb = [wpool.tile([128, 3 * HPC * D], F32R, name=f"w{k}")
                       for k in range(KT)]
                for k in range(KT):
                    nc.sync.dma_start(wsb[k][:], wqkv_in.ap()[k * 128:(k + 1) * 128, :])
                xT_v = xT_in.ap().rearrange("(k p) s -> p k s", p=128)
                for sc in range(SC):
                    xsc = xpool.tile([128, KT, 512], F32R, tag="xsc")
                    nc.sync.dma_start(xsc[:], xT_v[:, :, sc * 512:(sc + 1) * 512])
                    for t in range(NT):
                        pt = qkv_ps.tile([128, 512], F32, tag="qkvmm")
                        for k in range(KT):
                            nc.tensor.matmul(
                                pt[:],
                                wsb[k][:, t * 128:(t + 1) * 128],
                                xsc[:, k, :],
                                start=(k == 0), stop=(k == KT - 1))
                        nc.scalar.activation(
                            qkv_dst[t][:, sc * 512:(sc + 1) * 512], pt[:],
                            AF.Identity, bias=bsc[:, t:t + 1],
                            scale=0.125 if t < 2 else 1.0)

                # ---- V: vT -> V[h] ([keys, d] layout) ----------------------
                for vp in range(2):
                    for c4 in range(0, NB // 2, 4):
                        tp = tr_ps.tile([128, 512], BF16, tag="vtr")
                        for u in range(4):
                            c = c4 + u
                            nc.tensor.transpose(tp[:, u * 128:(u + 1) * 128],
                                                vT[vp][:, c * 128:(c + 1) * 128],
                                                idb[:])
                        for lh in range(2):
                            src = tp[:, 0:512].rearrange("p (u x) -> p u x", x=128)[
                                :, :, lh * 64:(lh + 1) * 64]
                            dst = V[2 * vp + lh][:, c4 * 64:(c4 + 4) * 64].rearrange(
                                "p (u d) -> p u d", d=64)
                            if lh == 0:
                                nc.scalar.copy(dst, src)
                            else:
                                nc.vector.tensor_copy(dst, src)

            # ---- block-diagonal stationaries via SBUF->SBUF DMA ------------
            for hp in range(2):
                kd = kdiag[hp][:, :]
                for lh, (p0, c0) in enumerate(((0, 0), (64, 64))):
                    dst = kd[p0:p0 + 64, :].rearrange("p (j c) -> p j c", c=128)[
                        :, :, c0:c0 + 64]
                    src = kT[hp][p0:p0 + 64, :].rearrange("p (j c) -> p j c", c=64)
                    nc.sync.dma_start(dst, src)
                vd = Vdiag[hp][:, :]
                for lh in range(2):
                    h = 2 * hp + lh
                    pd, cd = (0, 0) if lh == 0 else (64, 64)
                    for par in range(2):   # even j at V parts 0:64, odd at 64:128
                        dst = vd[pd:pd + 64, :].rearrange(
                            "p (c x) -> p c x", x=256)[:, :, par * 128 + cd:par * 128 + cd + 64]
                        src = V[h][par * 64:(par + 1) * 64, :].rearrange(
                            "p (c d) -> p c d", d=64)
                        nc.sync.dma_start(dst, src)

            # ---- attention, j-major, per head pair -------------------------
            with tc.tile_pool(name="sc_ps", bufs=2, space="PSUM") as sc_ps, \
                 tc.tile_pool(name="dn_ps", bufs=2, space="PSUM") as dn_ps, \
                 tc.tile_pool(name="ot_ps", bufs=1, space="PSUM") as ot_ps, \
                 tc.tile_pool(name="ex", bufs=4) as expool, \
                 tc.tile_pool(name="rc", bufs=4) as rcpool, \
                 tc.tile_pool(name="at", bufs=4) as atpool:
                for hp in range(2):
                    otp2 = ot_ps.tile([128, S], F32, tag="otp")
                    for ci, (j, group) in enumerate(chunks):
                        ncols = sum(n for _, _, n in group) * 64
                        spt = sc_ps.tile([128, 512], F32, tag="spt")
                        for gi, (pos, i0, n) in enumerate(group):
                            nc.tensor.matmul(
                                spt[:, pos * 64:(pos + n) * 64],
                                kdiag[hp][:, j * 128:(j + 1) * 128],
                                qT[hp][:, i0 * 64:(i0 + n) * 64],
                                start=(gi == 0), stop=(gi == len(group) - 1))
                        ex = expool.tile([128, 512], BF16, tag="ex")
                        nc.scalar.activation(ex[:, 0:ncols], spt[:, 0:ncols], AF.Exp)
                        dnb = dn_ps.tile([128, 512], F32, tag="dnb")
                        nc.tensor.matmul(dnb[:, 0:ncols], onesdiag[:],
                                         ex[:, 0:ncols], start=True, stop=True)
                        rec = rcpool.tile([128, 512], F32, tag="rec")
                        nc.vector.reciprocal_approx_fast(rec[:, 0:ncols],
                                                         dnb[:, 0:ncols])
                        at2 = atpool.tile([128, 512], BF16, tag="at2")
                        if ci % 3 == 2:
                            nc.vector.tensor_tensor(at2[:, 0:ncols], ex[:, 0:ncols],
                                                    rec[:, 0:ncols], ALU.mult)
                        else:
                            nc.gpsimd.tensor_tensor(at2[:, 0:ncols], ex[:, 0:ncols],
                                                    rec[:, 0:ncols], ALU.mult)
                        for pos, i0, n in group:
                            st, sp = flags[(j, i0)]
                            nc.tensor.matmul(
                                otp2[:, i0 * 64:(i0 + n) * 64],
                                Vdiag[hp][:, j * 128:(j + 1) * 128],
                                at2[:, pos * 64:(pos + n) * 64],
                                start=st, stop=sp)
                    for q in range(4):
                        if q % 2 == 0:
                            nc.scalar.copy(outSB[hp][:, q * 512:(q + 1) * 512],
                                           otp2[:, q * 512:(q + 1) * 512])
                        else:
                            nc.vector.tensor_copy(outSB[hp][:, q * 512:(q + 1) * 512],
                                                  otp2[:, q * 512:(q + 1) * 512])

            # ---- Wo projection (f32r), bias added on host ------------------
            with tc.tile_pool(name="yt", bufs=3) as ypool, \
                 tc.tile_pool(name="wo_ps", bufs=2, space="PSUM") as wo_ps:
                for st_ in range(ST):
                    yt = ypool.tile([128, E], F32, tag="yt")
                    for nchk in range(2):
                        pt = wo_ps.tile([128, 512], F32, tag="womm")
                        for hp in range(2):
                            nc.tensor.matmul(
                                pt[:],
                                outSB[hp][:, st_ * 128:(st_ + 1) * 128],
                                wo_sb[hp][:, nchk * 512:(nchk + 1) * 512],
                                start=(hp == 0), stop=(hp == 1))
                        if nchk == 0:
                            nc.scalar.copy(yt[:, nchk * 512:(nchk + 1) * 512], pt[:])
                        else:
                            nc.vector.tensor_copy(yt[:, nchk * 512:(nchk + 1) * 512],
                                                  pt[:])
                    nc.sync.dma_start(y_out.ap()[st_ * 128:(st_ + 1) * 128, :], yt[:])

    nc.compile()
    return nc


# ---------------------------------------------------------------- entry point

def kernel(x, Wq, bq, Wk, bk, Wv, bv, Wo, bo, block_rows, block_cols):
    global LAST_RESULTS
    from concourse.bass_utils import run_bass_kernel_spmd
    import os

    x = np.asarray(x, dtype=np.float32)
    Wq, Wk, Wv, Wo = (np.asarray(a, dtype=np.float32) for a in (Wq, Wk, Wv, Wo))
    bq, bk, bv, bo = (np.asarray(a, dtype=np.float32) for a in (bq, bk, bv, bo))

    plan = _plan(block_rows, block_cols)
    nc = _build_program(plan)

    xT = [np.ascontiguousarray(x[b].T) for b in range(B)]
    in_maps = []
    for c in range(NCORES):
        b, g = c // 4, c % 4
        cs = slice(g * HPC * D, (g + 1) * HPC * D)
        w_qkv = np.ascontiguousarray(
            np.concatenate([Wq[:, cs], Wk[:, cs], Wv[:, cs]], axis=1))
        b_qkv = np.ascontiguousarray(
            np.concatenate([bq[cs], bk[cs], bv[cs]]))
        w_o = np.ascontiguousarray(Wo[cs, :])
        in_maps.append(dict(xT_local=xT[b], w_qkv=w_qkv, b_qkv=b_qkv, w_o=w_o))

    trace = bool(int(os.environ.get("KERNEL_TRACE", "0")))
    res = run_bass_kernel_spmd(nc, in_maps, core_ids=list(range(NCORES)),
                               trace=trace)
    LAST_RESULTS = res

    y = np.zeros((B, S, E), dtype=np.float32)
    for c in range(NCORES):
        y[c // 4] += res.results[c]["y_partial"]
    y += bo
    return y


# revision 10
# speedup vs baseline: 2.7169x; 1.0381x over previous
"""Block-sparse attention Trainium2 kernel (8 NeuronCores, SPMD).

Sharding: data-parallel over (batch, head-group): core c handles batch b=c//4
and heads [4*(c%4) .. 4*(c%4)+4). Block index lists are replicated (used
host-side to build the static program). Each core returns a partial
[S, E] output (its heads' contribution through Wo); the host sums the 4
partials per batch and adds bo once.

Design (transposed-scores dataflow, j-major):
  Host feeds x^T; QKV projection runs weights-stationary in float32r
  (1 cyc/row at 512-wide moving) producing q^T,k^T (bf16, q pre-scaled
  by 1/sqrt(D)) and v^T (bf16). V is put in [keys, d] layout by PE
  transposes. Per head pair, block-diagonal stationaries are built by
  SBUF->SBUF DMA:
    kdiag_j = [[kT_A(j), 0], [0, kT_B(j)]]   (dA|dB x keysA|keysB)
    Vdiag_j = [[V_A(j), 0], [0, V_B(j)]]     (keysA|keysB x dA|dB)
  For each col-block j and chunk of row-blocks i (<=8 blocks, 512 cols):
    scoresT = kdiag_j^T @ qT[:, i-cols]   -> PSUM [128=keysA|keysB, cols]
    expT    = ACT exp -> SBUF bf16
    denB    = onesdiag^T @ expT           -> per-head key-sums replicated
                                             across that head's 64 partitions
    att2T   = expT / denB                 (DVE divide, elementwise)
    out^T  += Vdiag_j^T @ att2T           -> PSUM otp2 [128=dA|dB, S]
  No attn transposes, no reduce, no recip, no attnT copies.
  Wo projection in float32r from out^T tiles; bias added on host.
"""
import numpy as np

B, S, E, H, D, BS = 2, 2048, 1024, 16, 64, 64
NB = S // BS          # 32
NCORES = 8
HPC = 4               # heads per core

LAST_RESULTS = None   # BassKernelResults of the most recent run (for test.py)

# chunk indices (global, per head-pair) whose divide runs on gpsimd instead
# of DVE; tune for engine balance.
DIV_GPSIMD_MOD = 0    # 0 = all on DVE; k>0 = every k-th chunk on gpsimd


# ---------------------------------------------------------------- host planning

def _plan(block_rows, block_cols):
    """j-major static schedule shared by every head-pair/core.

    For each col-block j: active row-blocks i, split into maximal
    consecutive runs that do not cross multiples of 8 (PSUM bank alignment
    for the out^T accumulator), grouped into chunks of <=8 blocks
    (<=512 cols, one PSUM bank per scores/den tile).

    Returns:
      chunks: list of (j, [(pos, i0, n), ...]) in emission order; pos is the
              64-col block offset inside the chunk tile.
      flags:  dict (j, i0) -> [start, stop] for the otp2 accumulation.
    """
    mask = np.zeros((NB, NB), dtype=bool)
    for r, c in zip(np.asarray(block_rows).tolist(), np.asarray(block_cols).tolist()):
        mask[int(r), int(c)] = True

    chunks = []
    pieces_by_bank = {}
    for j in range(NB):
        ilist = np.nonzero(mask[:, j])[0].tolist()
        runs = []
        cur = None
        for i in ilist:
            if cur is not None and i == cur[0] + cur[1] and (i % 8 != 0):
                cur[1] += 1
            else:
                cur = [i, 1]
                runs.append(cur)
        # group runs into chunks of <=8 blocks
        group, nb_ = [], 0
        for i0, n in runs:
            if nb_ + n > 8:
                chunks.append((j, group))
                group, nb_ = [], 0
            group.append((nb_, i0, n))
            nb_ += n
        if group:
            chunks.append((j, group))
    # otp2 start/stop: first/last piece per 8-i bank in emission order
    order = []
    for j, group in chunks:
        for pos, i0, n in group:
            order.append((j, i0, n))
    flags = {}
    first_seen, last_seen = {}, {}
    for idx, (j, i0, n) in enumerate(order):
        bk = i0 // 8
        assert (i0 + n - 1) // 8 == bk
        if bk not in first_seen:
            first_seen[bk] = (j, i0)
        last_seen[bk] = (j, i0)
        flags[(j, i0)] = [False, False]
    for bk, key in first_seen.items():
        flags[key][0] = True
    for bk, key in last_seen.items():
        flags[key][1] = True
    return dict(chunks=chunks, flags=flags)


# ---------------------------------------------------------------- bass program

def _build_program(plan):
    import concourse.bacc as bacc
    import concourse.mybir as mybir
    from concourse.tile import TileContext
    from concourse import masks

    F32 = mybir.dt.float32
    F32R = mybir.dt.float32r
    BF16 = mybir.dt.bfloat16
    AF = mybir.ActivationFunctionType
    ALU = mybir.AluOpType

    nc = bacc.Bacc("TRN2", target_bir_lowering=False, debug=False)

    xT_in = nc.dram_tensor("xT_local", [E, S], BF16, kind="ExternalInput")
    wqkv_in = nc.dram_tensor("w_qkv", [E, 3 * HPC * D], BF16, kind="ExternalInput")
    bqkv_in = nc.dram_tensor("b_qkv", [3 * HPC * D], F32, kind="ExternalInput")
    wo_in = nc.dram_tensor("w_o", [HPC * D, E], F32R, kind="ExternalInput")
    y_out = nc.dram_tensor("y_partial", [S, E], BF16, kind="ExternalOutput")

    NT = 3 * HPC * D // 128      # 6 qkv n-tiles
    KT = E // 128                # 8 contraction tiles
    ST = S // 128                # 16 s tiles
    SC = S // 512                # 4 s-chunks

    chunks, flags = plan['chunks'], plan['flags']

    with TileContext(nc) as tc:
        with tc.tile_pool(name="const", bufs=1) as cpool, \
             tc.tile_pool(name="qk", bufs=1) as qkpool, \
             tc.tile_pool(name="vt", bufs=1) as vtpool, \
             tc.tile_pool(name="diag", bufs=1) as dgpool, \
             tc.tile_pool(name="outsb", bufs=1) as opool, \
             tc.tile_pool(name="wo", bufs=1) as wop:

            idb = cpool.tile([128, 128], BF16)
            masks.make_identity(nc, idb[:])
            bqkv_sb = cpool.tile([128, NT], F32)
            nc.sync.dma_start(bqkv_sb[:], bqkv_in.ap().rearrange("(t p) -> p t", p=128))
            bsc = cpool.tile([128, NT], F32)
            nc.scalar.mul(bsc[:, 0:2], bqkv_sb[:, 0:2], 0.125)
            nc.scalar.copy(bsc[:, 2:NT], bqkv_sb[:, 2:NT])
            onesdiag = cpool.tile([128, 128], BF16)
            nc.gpsimd.memset(onesdiag[:], 0.0)
            nc.gpsimd.memset(onesdiag[0:64, 0:64], 1.0)
            nc.gpsimd.memset(onesdiag[64:128, 64:128], 1.0)

            wo_sb = [wop.tile([128, E], F32R, name=f"wo{hp}") for hp in range(2)]

            # q0,q1,k0,k1 bf16 [128, S]; vT per pair bf16
            qT = [qkpool.tile([128, S], BF16, name=f"qT{hp}") for hp in range(2)]
            kT = [qkpool.tile([128, S], BF16, name=f"kT{hp}") for hp in range(2)]
            vT = [vtpool.tile([128, S], BF16, name=f"vT{hp}") for hp in range(2)]
            V = [vtpool.tile([128, (NB // 2) * D], BF16, name=f"V{h}") for h in range(HPC)]
            kdiag = [dgpool.tile([128, NB * 128], BF16, name=f"kdiag{hp}") for hp in range(2)]
            Vdiag = [dgpool.tile([128, NB * 128], BF16, name=f"Vdiag{hp}") for hp in range(2)]
            for hp in range(2):
                nc.gpsimd.memset(kdiag[hp][:], 0.0)
                nc.gpsimd.memset(Vdiag[hp][:], 0.0)
            outSB = [opool.tile([128, S], F32R, name=f"outSB{hp}") for hp in range(2)]

            # ---- QKV projection (weights stationary, f32r) -----------------
            qkv_dst = [qT[0], qT[1], kT[0], kT[1], vT[0], vT[1]]
            with tc.tile_pool(name="xin", bufs=3) as xpool, \
                 tc.tile_pool(name="wq", bufs=1) as wpool, \
                 tc.tile_pool(name="qkv_ps", bufs=4, space="PSUM") as qkv_ps, \
                 tc.tile_pool(name="tr_ps", bufs=2, space="PSUM") as tr_ps:
                wsb = [wpool.tile([128, 3 * HPC * D], BF16, name=f"w{k}")
                       for k in range(KT)]
                for k in range(KT):
                    nc.sync.dma_start(wsb[k][:], wqkv_in.ap()[k * 128:(k + 1) * 128, :])
                xT_v = xT_in.ap().rearrange("(k p) s -> p k s", p=128)
                for sc in range(SC):
                    xsc = xpool.tile([128, KT, 512], BF16, tag="xsc")
                    nc.sync.dma_start(xsc[:], xT_v[:, :, sc * 512:(sc + 1) * 512])
                    for t in range(NT):
                        pt = qkv_ps.tile([128, 512], F32, tag="qkvmm")
                        for k in range(KT):
                            nc.tensor.matmul(
                                pt[:],
                                wsb[k][:, t * 128:(t + 1) * 128],
                                xsc[:, k, :],
                                start=(k == 0), stop=(k == KT - 1))
                        nc.scalar.activation(
                            qkv_dst[t][:, sc * 512:(sc + 1) * 512], pt[:],
                            AF.Identity, bias=bsc[:, t:t + 1],
                            scale=0.125 if t < 2 else 1.0)

                # ---- V: vT -> V[h] ([keys, d] layout) ----------------------
                for vp in range(2):
                    for c4 in range(0, NB // 2, 4):
                        tp = tr_ps.tile([128, 512], BF16, tag="vtr")
                        for u in range(4):
                            c = c4 + u
                            nc.tensor.transpose(tp[:, u * 128:(u + 1) * 128],
                                                vT[vp][:, c * 128:(c + 1) * 128],
                                                idb[:])
                        for lh in range(2):
                            src = tp[:, 0:512].rearrange("p (u x) -> p u x", x=128)[
                                :, :, lh * 64:(lh + 1) * 64]
                            dst = V[2 * vp + lh][:, c4 * 64:(c4 + 4) * 64].rearrange(
                                "p (u d) -> p u d", d=64)
                            if lh == 0:
                                nc.scalar.copy(dst, src)
                            else:
                                nc.vector.tensor_copy(dst, src)

            for hp in range(2):
                nc.sync.dma_start(wo_sb[hp][:],
                                  wo_in.ap()[hp * 128:(hp + 1) * 128, :])

            # ---- block-diagonal stationaries via SBUF->SBUF DMA ------------
            for hp in range(2):
                kd = kdiag[hp][:, :]
                for lh, (p0, c0) in enumerate(((0, 0), (64, 64))):
                    dst = kd[p0:p0 + 64, :].rearrange("p (j c) -> p j c", c=128)[
                        :, :, c0:c0 + 64]
                    src = kT[hp][p0:p0 + 64, :].rearrange("p (j c) -> p j c", c=64)
                    nc.sync.dma_start(dst, src)
                vd = Vdiag[hp][:, :]
                for lh in range(2):
                    h = 2 * hp + lh
                    pd, cd = (0, 0) if lh == 0 else (64, 64)
                    for par in range(2):   # even j at V parts 0:64, odd at 64:128
                        dst = vd[pd:pd + 64, :].rearrange(
                            "p (c x) -> p c x", x=256)[:, :, par * 128 + cd:par * 128 + cd + 64]
                        src = V[h][par * 64:(par + 1) * 64, :].rearrange(
                            "p (c d) -> p c d", d=64)
                        nc.sync.dma_start(dst, src)

            # ---- attention, j-major, per head pair -------------------------
            with tc.tile_pool(name="sc_ps", bufs=2, space="PSUM") as sc_ps, \
                 tc.tile_pool(name="dn_ps", bufs=2, space="PSUM") as dn_ps, \
                 tc.tile_pool(name="ot_ps", bufs=1, space="PSUM") as ot_ps, \
                 tc.tile_pool(name="ex", bufs=4) as expool, \
                 tc.tile_pool(name="rc", bufs=4) as rcpool, \
                 tc.tile_pool(name="at", bufs=4) as atpool:
                for hp in range(2):
                    otp2 = ot_ps.tile([128, S], F32, tag="otp")
                    for ci, (j, group) in enumerate(chunks):
                        ncols = sum(n for _, _, n in group) * 64
                        spt = sc_ps.tile([128, 512], F32, tag="spt")
                        for gi, (pos, i0, n) in enumerate(group):
                            nc.tensor.matmul(
                                spt[:, pos * 64:(pos + n) * 64],
                                kdiag[hp][:, j * 128:(j + 1) * 128],
                                qT[hp][:, i0 * 64:(i0 + n) * 64],
                                start=(gi == 0), stop=(gi == len(group) - 1))
                        ex = expool.tile([128, 512], BF16, tag="ex")
                        nc.scalar.activation(ex[:, 0:ncols], spt[:, 0:ncols], AF.Exp)
                        dnb = dn_ps.tile([128, 512], F32, tag="dnb")
                        nc.tensor.matmul(dnb[:, 0:ncols], onesdiag[:],
                                         ex[:, 0:ncols], start=True, stop=True)
                        rec = rcpool.tile([128, 512], F32, tag="rec")
                        nc.vector.reciprocal_approx_fast(rec[:, 0:ncols],
                                                         dnb[:, 0:ncols])
                        at2 = atpool.tile([128, 512], BF16, tag="at2")
                        if ci % 3 == 2:
                            nc.vector.tensor_tensor(at2[:, 0:ncols], ex[:, 0:ncols],
                                                    rec[:, 0:ncols], ALU.mult)
                        else:
                            nc.gpsimd.tensor_tensor(at2[:, 0:ncols], ex[:, 0:ncols],
                                                    rec[:, 0:ncols], ALU.mult)
                        for pos, i0, n in group:
                            st, sp = flags[(j, i0)]
                            nc.tensor.matmul(
                                otp2[:, i0 * 64:(i0 + n) * 64],
                                Vdiag[hp][:, j * 128:(j + 1) * 128],
                                at2[:, pos * 64:(pos + n) * 64],
                                start=st, stop=sp)
                    for q in range(4):
                        if q % 2 == 0:
                            nc.scalar.copy(outSB[hp][:, q * 512:(q + 1) * 512],
                                           otp2[:, q * 512:(q + 1) * 512])
                        else:
                            nc.vector.tensor_copy(outSB[hp][:, q * 512:(q + 1) * 512],
                                                  otp2[:, q * 512:(q + 1) * 512])

            # ---- Wo projection (f32r), bias added on host ------------------
            with tc.tile_pool(name="yt", bufs=3) as ypool, \
                 tc.tile_pool(name="wo_ps", bufs=2, space="PSUM") as wo_ps:
                for st_ in range(ST):
                    yt = ypool.tile([128, E], BF16, tag="yt")
                    for nchk in range(2):
                        pt = wo_ps.tile([128, 512], F32, tag="womm")
                        for hp in range(2):
                            nc.tensor.matmul(
                                pt[:],
                                outSB[hp][:, st_ * 128:(st_ + 1) * 128],
                                wo_sb[hp][:, nchk * 512:(nchk + 1) * 512],
                                start=(hp == 0), stop=(hp == 1))
                        if nchk == 0:
                            nc.scalar.copy(yt[:, nchk * 512:(nchk + 1) * 512], pt[:])
                        else:
                            nc.vector.tensor_copy(yt[:, nchk * 512:(nchk + 1) * 512],
                                                  pt[:])
                    nc.sync.dma_start(y_out.ap()[st_ * 128:(st_ + 1) * 128, :], yt[:])

    nc.compile()
    return nc


# ---------------------------------------------------------------- entry point

def kernel(x, Wq, bq, Wk, bk, Wv, bv, Wo, bo, block_rows, block_cols):
    global LAST_RESULTS
    from concourse.bass_utils import run_bass_kernel_spmd
    import os

    x = np.asarray(x, dtype=np.float32)
    Wq, Wk, Wv, Wo = (np.asarray(a, dtype=np.float32) for a in (Wq, Wk, Wv, Wo))
    bq, bk, bv, bo = (np.asarray(a, dtype=np.float32) for a in (bq, bk, bv, bo))

    plan = _plan(block_rows, block_cols)
    nc = _build_program(plan)

    import ml_dtypes
    bf16 = ml_dtypes.bfloat16
    xT = [np.ascontiguousarray(x[b].T).astype(bf16) for b in range(B)]
    in_maps = []
    for c in range(NCORES):
        b, g = c // 4, c % 4
        cs = slice(g * HPC * D, (g + 1) * HPC * D)
        w_qkv = np.ascontiguousarray(
            np.concatenate([Wq[:, cs], Wk[:, cs], Wv[:, cs]], axis=1)).astype(bf16)
        b_qkv = np.ascontiguousarray(
            np.concatenate([bq[cs], bk[cs], bv[cs]]))
        w_o = np.ascontiguousarray(Wo[cs, :])
        in_maps.append(dict(xT_local=xT[b], w_qkv=w_qkv, b_qkv=b_qkv, w_o=w_o))

    trace = bool(int(os.environ.get("KERNEL_TRACE", "0")))
    res = run_bass_kernel_spmd(nc, in_maps, core_ids=list(range(NCORES)),
                               trace=trace)
    LAST_RESULTS = res

    y = np.zeros((B, S, E), dtype=np.float32)
    for c in range(NCORES):
        y[c // 4] += np.asarray(res.results[c]["y_partial"], dtype=np.float32)
    y += bo
    return y


# revision 12
# speedup vs baseline: 2.7456x; 1.0106x over previous
"""Block-sparse attention Trainium2 kernel (8 NeuronCores, SPMD).

Sharding: data-parallel over (batch, head-group): core c handles batch b=c//4
and heads [4*(c%4) .. 4*(c%4)+4). Block index lists are replicated (used
host-side to build the static program). Each core returns a partial
[S, E] output (its heads' contribution through Wo); the host sums the 4
partials per batch and adds bo once.

Design (transposed-scores dataflow, j-major):
  Host feeds x^T; QKV projection runs weights-stationary in float32r
  (1 cyc/row at 512-wide moving) producing q^T,k^T (bf16, q pre-scaled
  by 1/sqrt(D)) and v^T (bf16). V is put in [keys, d] layout by PE
  transposes. Per head pair, block-diagonal stationaries are built by
  SBUF->SBUF DMA:
    kdiag_j = [[kT_A(j), 0], [0, kT_B(j)]]   (dA|dB x keysA|keysB)
    Vdiag_j = [[V_A(j), 0], [0, V_B(j)]]     (keysA|keysB x dA|dB)
  For each col-block j and chunk of row-blocks i (<=8 blocks, 512 cols):
    scoresT = kdiag_j^T @ qT[:, i-cols]   -> PSUM [128=keysA|keysB, cols]
    expT    = ACT exp -> SBUF bf16
    denB    = onesdiag^T @ expT           -> per-head key-sums replicated
                                             across that head's 64 partitions
    att2T   = expT / denB                 (DVE divide, elementwise)
    out^T  += Vdiag_j^T @ att2T           -> PSUM otp2 [128=dA|dB, S]
  No attn transposes, no reduce, no recip, no attnT copies.
  Wo projection in float32r from out^T tiles; bias added on host.
"""
import numpy as np

B, S, E, H, D, BS = 2, 2048, 1024, 16, 64, 64
NB = S // BS          # 32
NCORES = 8
HPC = 4               # heads per core

LAST_RESULTS = None   # BassKernelResults of the most recent run (for test.py)

# chunk indices (global, per head-pair) whose divide runs on gpsimd instead
# of DVE; tune for engine balance.
DIV_GPSIMD_MOD = 0    # 0 = all on DVE; k>0 = every k-th chunk on gpsimd


# ---------------------------------------------------------------- host planning

def _plan(block_rows, block_cols):
    """j-major static schedule shared by every head-pair/core.

    For each col-block j: active row-blocks i, split into maximal
    consecutive runs that do not cross multiples of 8 (PSUM bank alignment
    for the out^T accumulator), grouped into chunks of <=8 blocks
    (<=512 cols, one PSUM bank per scores/den tile).

    Returns:
      chunks: list of (j, [(pos, i0, n), ...]) in emission order; pos is the
              64-col block offset inside the chunk tile.
      flags:  dict (j, i0) -> [start, stop] for the otp2 accumulation.
    """
    mask = np.zeros((NB, NB), dtype=bool)
    for r, c in zip(np.asarray(block_rows).tolist(), np.asarray(block_cols).tolist()):
        mask[int(r), int(c)] = True

    chunks = []
    pieces_by_bank = {}
    for j in range(NB):
        ilist = np.nonzero(mask[:, j])[0].tolist()
        runs = []
        cur = None
        for i in ilist:
            if cur is not None and i == cur[0] + cur[1] and (i % 8 != 0):
                cur[1] += 1
            else:
                cur = [i, 1]
                runs.append(cur)
        # group runs into chunks of <=8 blocks
        group, nb_ = [], 0
        for i0, n in runs:
            if nb_ + n > 8:
                chunks.append((j, group))
                group, nb_ = [], 0
            group.append((nb_, i0, n))
            nb_ += n
        if group:
            chunks.append((j, group))
    # otp2 start/stop: first/last piece per 8-i bank in emission order
    order = []
    for j, group in chunks:
        for pos, i0, n in group:
            order.append((j, i0, n))
    flags = {}
    first_seen, last_seen = {}, {}
    for idx, (j, i0, n) in enumerate(order):
        bk = i0 // 8
        assert (i0 + n - 1) // 8 == bk
        if bk not in first_seen:
            first_seen[bk] = (j, i0)
        last_seen[bk] = (j, i0)
        flags[(j, i0)] = [False, False]
    for bk, key in first_seen.items():
        flags[key][0] = True
    for bk, key in last_seen.items():
        flags[key][1] = True
    return dict(chunks=chunks, flags=flags)


# ---------------------------------------------------------------- bass program

def _build_program(plan):
    import concourse.bacc as bacc
    import concourse.mybir as mybir
    from concourse.tile import TileContext
    from concourse import masks

    F32 = mybir.dt.float32
    F32R = mybir.dt.float32r
    BF16 = mybir.dt.bfloat16
    AF = mybir.ActivationFunctionType
    ALU = mybir.AluOpType

    nc = bacc.Bacc("TRN2", target_bir_lowering=False, debug=False)

    xT_in = nc.dram_tensor("xT_local", [E, S], BF16, kind="ExternalInput")
    wqkv_in = nc.dram_tensor("w_qkv", [E, 3 * HPC * D], BF16, kind="ExternalInput")
    bqkv_in = nc.dram_tensor("b_qkv", [3 * HPC * D], F32, kind="ExternalInput")
    wo_in = nc.dram_tensor("w_o", [HPC * D, E], F32R, kind="ExternalInput")
    y_out = nc.dram_tensor("y_partial", [S, E], BF16, kind="ExternalOutput")

    NT = 3 * HPC * D // 128      # 6 qkv n-tiles
    KT = E // 128                # 8 contraction tiles
    ST = S // 128                # 16 s tiles
    SC = S // 512                # 4 s-chunks

    chunks, flags = plan['chunks'], plan['flags']

    with TileContext(nc) as tc:
        with tc.tile_pool(name="const", bufs=1) as cpool, \
             tc.tile_pool(name="qk", bufs=1) as qkpool, \
             tc.tile_pool(name="vt", bufs=1) as vtpool, \
             tc.tile_pool(name="diag", bufs=1) as dgpool, \
             tc.tile_pool(name="outsb", bufs=1) as opool, \
             tc.tile_pool(name="wo", bufs=1) as wop:

            idb = cpool.tile([128, 128], BF16)
            masks.make_identity(nc, idb[:])
            bqkv_sb = cpool.tile([128, NT], F32)
            nc.sync.dma_start(bqkv_sb[:], bqkv_in.ap().rearrange("(t p) -> p t", p=128))
            bsc = cpool.tile([128, NT], F32)
            nc.scalar.mul(bsc[:, 0:2], bqkv_sb[:, 0:2], 0.125)
            nc.scalar.copy(bsc[:, 2:NT], bqkv_sb[:, 2:NT])
            onesdiag = cpool.tile([128, 128], BF16)
            nc.gpsimd.memset(onesdiag[:], 0.0)
            nc.gpsimd.memset(onesdiag[0:64, 0:64], 1.0)
            nc.gpsimd.memset(onesdiag[64:128, 64:128], 1.0)

            wo_sb = [wop.tile([128, E], F32R, name=f"wo{hp}") for hp in range(2)]

            # q0,q1,k0,k1 bf16 [128, S]; vT per pair bf16
            qT = [qkpool.tile([128, S], BF16, name=f"qT{hp}") for hp in range(2)]
            kT = [qkpool.tile([128, S], BF16, name=f"kT{hp}") for hp in range(2)]
            vT = [vtpool.tile([128, S], BF16, name=f"vT{hp}") for hp in range(2)]
            V = [vtpool.tile([128, (NB // 2) * D], BF16, name=f"V{h}") for h in range(HPC)]
            kdiag = [dgpool.tile([128, NB * 128], BF16, name=f"kdiag{hp}") for hp in range(2)]
            Vdiag = [dgpool.tile([128, NB * 128], BF16, name=f"Vdiag{hp}") for hp in range(2)]
            for hp in range(2):
                nc.vector.memset(kdiag[hp][:], 0.0)
                nc.vector.memset(Vdiag[hp][:], 0.0)
            outSB = [opool.tile([128, S], F32R, name=f"outSB{hp}") for hp in range(2)]

            # ---- QKV projection (weights stationary, f32r) -----------------
            qkv_dst = [qT[0], qT[1], kT[0], kT[1], vT[0], vT[1]]
            with tc.tile_pool(name="xin", bufs=3) as xpool, \
                 tc.tile_pool(name="wq", bufs=1) as wpool, \
                 tc.tile_pool(name="qkv_ps", bufs=4, space="PSUM") as qkv_ps, \
                 tc.tile_pool(name="tr_ps", bufs=2, space="PSUM") as tr_ps:
                wsb = [wpool.tile([128, 3 * HPC * D], BF16, name=f"w{k}")
                       for k in range(KT)]
                for k in range(KT):
                    nc.sync.dma_start(wsb[k][:], wqkv_in.ap()[k * 128:(k + 1) * 128, :])
                xT_v = xT_in.ap().rearrange("(k p) s -> p k s", p=128)
                xsc = [xpool.tile([128, KT, 512], BF16, name=f"xsc{sc}")
                       for sc in range(SC)]
                for sc in range(SC):
                    nc.sync.dma_start(xsc[sc][:], xT_v[:, :, sc * 512:(sc + 1) * 512])

                def emit_kdiag(hp):
                    kd = kdiag[hp][:, :]
                    for (p0, c0) in ((0, 0), (64, 64)):
                        dst = kd[p0:p0 + 64, :].rearrange("p (j c) -> p j c", c=128)[
                            :, :, c0:c0 + 64]
                        src = kT[hp][p0:p0 + 64, :].rearrange("p (j c) -> p j c", c=64)
                        nc.sync.dma_start(dst, src)

                def emit_vprep(vp):
                    for c4 in range(0, NB // 2, 4):
                        tp = tr_ps.tile([128, 512], BF16, tag="vtr")
                        for u in range(4):
                            c = c4 + u
                            nc.tensor.transpose(tp[:, u * 128:(u + 1) * 128],
                                                vT[vp][:, c * 128:(c + 1) * 128],
                                                idb[:])
                        for lh in range(2):
                            src = tp[:, 0:512].rearrange("p (u x) -> p u x", x=128)[
                                :, :, lh * 64:(lh + 1) * 64]
                            dst = V[2 * vp + lh][:, c4 * 64:(c4 + 4) * 64].rearrange(
                                "p (u d) -> p u d", d=64)
                            if lh == 0:
                                nc.scalar.copy(dst, src)
                            else:
                                nc.vector.tensor_copy(dst, src)
                    vd = Vdiag[vp][:, :]
                    for lh in range(2):
                        h = 2 * vp + lh
                        pd, cd = (0, 0) if lh == 0 else (64, 64)
                        for par in range(2):   # even j at V parts 0:64, odd 64:128
                            dst = vd[pd:pd + 64, :].rearrange(
                                "p (c x) -> p c x", x=256)[
                                :, :, par * 128 + cd:par * 128 + cd + 64]
                            src = V[h][par * 64:(par + 1) * 64, :].rearrange(
                                "p (c d) -> p c d", d=64)
                            nc.sync.dma_start(dst, src)

                for t in (0, 2, 4, 1, 3, 5):
                    for sc in range(SC):
                        pt = qkv_ps.tile([128, 512], F32, tag="qkvmm")
                        for k in range(KT):
                            nc.tensor.matmul(
                                pt[:],
                                wsb[k][:, t * 128:(t + 1) * 128],
                                xsc[sc][:, k, :],
                                start=(k == 0), stop=(k == KT - 1))
                        nc.scalar.activation(
                            qkv_dst[t][:, sc * 512:(sc + 1) * 512], pt[:],
                            AF.Identity, bias=bsc[:, t:t + 1],
                            scale=0.125 if t < 2 else 1.0)
                    if t == 2:
                        emit_kdiag(0)
                    elif t == 3:
                        emit_kdiag(1)
                    elif t == 4:
                        emit_vprep(0)
                    elif t == 5:
                        emit_vprep(1)

            for hp in range(2):
                nc.sync.dma_start(wo_sb[hp][:],
                                  wo_in.ap()[hp * 128:(hp + 1) * 128, :])

            # ---- attention, j-major, per head pair -------------------------
            with tc.tile_pool(name="sc_ps", bufs=2, space="PSUM") as sc_ps, \
                 tc.tile_pool(name="dn_ps", bufs=2, space="PSUM") as dn_ps, \
                 tc.tile_pool(name="ot_ps", bufs=1, space="PSUM") as ot_ps, \
                 tc.tile_pool(name="ex", bufs=4) as expool, \
                 tc.tile_pool(name="rc", bufs=4) as rcpool, \
                 tc.tile_pool(name="at", bufs=4) as atpool:
                for hp in range(2):
                    otp2 = ot_ps.tile([128, S], F32, tag="otp")
                    for ci, (j, group) in enumerate(chunks):
                        ncols = sum(n for _, _, n in group) * 64
                        spt = sc_ps.tile([128, 512], F32, tag="spt")
                        for gi, (pos, i0, n) in enumerate(group):
                            nc.tensor.matmul(
                                spt[:, pos * 64:(pos + n) * 64],
                                kdiag[hp][:, j * 128:(j + 1) * 128],
                                qT[hp][:, i0 * 64:(i0 + n) * 64],
                                start=(gi == 0), stop=(gi == len(group) - 1))
                        ex = expool.tile([128, 512], BF16, tag="ex")
                        nc.scalar.activation(ex[:, 0:ncols], spt[:, 0:ncols], AF.Exp)
                        dnb = dn_ps.tile([128, 512], F32, tag="dnb")
                        nc.tensor.matmul(dnb[:, 0:ncols], onesdiag[:],
                                         ex[:, 0:ncols], start=True, stop=True)
                        rec = rcpool.tile([128, 512], F32, tag="rec")
                        nc.vector.reciprocal_approx_fast(rec[:, 0:ncols],
                                                         dnb[:, 0:ncols])
                        at2 = atpool.tile([128, 512], BF16, tag="at2")
                        if ci % 3 == 2:
                            nc.vector.tensor_tensor(at2[:, 0:ncols], ex[:, 0:ncols],
                                                    rec[:, 0:ncols], ALU.mult)
                        else:
                            nc.gpsimd.tensor_tensor(at2[:, 0:ncols], ex[:, 0:ncols],
                                                    rec[:, 0:ncols], ALU.mult)
                        for pos, i0, n in group:
                            st, sp = flags[(j, i0)]
                            nc.tensor.matmul(
                                otp2[:, i0 * 64:(i0 + n) * 64],
                                Vdiag[hp][:, j * 128:(j + 1) * 128],
                                at2[:, pos * 64:(pos + n) * 64],
                                start=st, stop=sp)
                    for q in range(4):
                        if q % 2 == 0:
                            nc.scalar.copy(outSB[hp][:, q * 512:(q + 1) * 512],
                                           otp2[:, q * 512:(q + 1) * 512])
                        else:
                            nc.vector.tensor_copy(outSB[hp][:, q * 512:(q + 1) * 512],
                                                  otp2[:, q * 512:(q + 1) * 512])

            # ---- Wo projection (f32r), bias added on host ------------------
            with tc.tile_pool(name="yt", bufs=3) as ypool, \
                 tc.tile_pool(name="wo_ps", bufs=2, space="PSUM") as wo_ps:
                for st_ in range(ST):
                    yt = ypool.tile([128, E], BF16, tag="yt")
                    for nchk in range(2):
                        pt = wo_ps.tile([128, 512], F32, tag="womm")
                        for hp in range(2):
                            nc.tensor.matmul(
                                pt[:],
                                outSB[hp][:, st_ * 128:(st_ + 1) * 128],
                                wo_sb[hp][:, nchk * 512:(nchk + 1) * 512],
                                start=(hp == 0), stop=(hp == 1))
                        if nchk == 0:
                            nc.scalar.copy(yt[:, nchk * 512:(nchk + 1) * 512], pt[:])
                        else:
                            nc.vector.tensor_copy(yt[:, nchk * 512:(nchk + 1) * 512],
                                                  pt[:])
                    nc.sync.dma_start(y_out.ap()[st_ * 128:(st_ + 1) * 128, :], yt[:])

    nc.compile()
    return nc


# ---------------------------------------------------------------- entry point

def kernel(x, Wq, bq, Wk, bk, Wv, bv, Wo, bo, block_rows, block_cols):
    global LAST_RESULTS
    from concourse.bass_utils import run_bass_kernel_spmd
    import os

    x = np.asarray(x, dtype=np.float32)
    Wq, Wk, Wv, Wo = (np.asarray(a, dtype=np.float32) for a in (Wq, Wk, Wv, Wo))
    bq, bk, bv, bo = (np.asarray(a, dtype=np.float32) for a in (bq, bk, bv, bo))

    plan = _plan(block_rows, block_cols)
    nc = _build_program(plan)

    import ml_dtypes
    bf16 = ml_dtypes.bfloat16
    xT = [np.ascontiguousarray(x[b].T).astype(bf16) for b in range(B)]
    in_maps = []
    for c in range(NCORES):
        b, g = c // 4, c % 4
        cs = slice(g * HPC * D, (g + 1) * HPC * D)
        w_qkv = np.ascontiguousarray(
            np.concatenate([Wq[:, cs], Wk[:, cs], Wv[:, cs]], axis=1)).astype(bf16)
        b_qkv = np.ascontiguousarray(
            np.concatenate([bq[cs], bk[cs], bv[cs]]))
        w_o = np.ascontiguousarray(Wo[cs, :])
        in_maps.append(dict(xT_local=xT[b], w_qkv=w_qkv, b_qkv=b_qkv, w_o=w_o))

    trace = bool(int(os.environ.get("KERNEL_TRACE", "0")))
    res = run_bass_kernel_spmd(nc, in_maps, core_ids=list(range(NCORES)),
                               trace=trace)
    LAST_RESULTS = res

    y = np.zeros((B, S, E), dtype=np.float32)
    for c in range(NCORES):
        y[c // 4] += np.asarray(res.results[c]["y_partial"], dtype=np.float32)
    y += bo
    return y
